# revision 15
# baseline (speedup 1.0000x reference)
"""Trainium2 Bass kernel for nn_CrossAttention (B=4, C=256, H=W=64).

Sharding: 8 cores = (batch b, query-half h). Host permutes the N axis of
each core's inputs to LOCAL-half order (own query half first), so one SPMD
program serves all cores: queries = local columns [0, IH), and the key/j
loop runs local half 0 then half 1 (sum order is irrelevant).

Per core:
  q = Wq x_q + bq    [32, IH] stored 4x row-replicated as q4 [128, IH]
  k = Wk x_f         [32, N ] 4x row-replicated as k4 (bk dropped:
                     j-constant shift is softmax-invariant)
  vT = (Wv x_f)^T    [N, 256] bf16 (bv folded into bce on host)
  S^T[j, i] = k_j . q_i  (two K=32 row strips run concurrently on the PE)
  E = exp(S^T) bf16      (f32 exp is safe: |S| <~ 30)
  r[i] = sum_j E[j, i]   (ones-matmul, 2 col-packed M=1 strips)
  att = (vT^T E) / r ; comb = Wc [x_q; att] + bce ; out[i] = sum_c |comb|

Pipeline structure (the point of this version):
  - scores are issued 2 groups ahead so exp(g+1) runs while the PE does
    the attended matmuls of group g: the inner loop is PE-bound, the
    scalar engine runs back-to-back EXPs and nothing else.
  - the per-block tail never touches DRAM and keeps the PE warm: r strips
    fold on DVE, 1/r via reciprocal_approx_fast (DVE), partition-broadcast
    via a K=1 ones-matmul, |.| via DVE scalar_tensor_tensor, biases via
    K=1 matmuls; the abs-sum output matmuls are deferred into the next
    block's group stream.
  - PSUM: stp ring 2x[128,2,512] (also lends a half-tile to the c2=1
    combine accumulator), attp ring 2, rp ring 2x[128,512] hosting
    rp/cp0/rbc/outw per block. 8 banks exactly.
"""

import numpy as np
import ml_dtypes

import concourse.bass as bass
import concourse.bacc as bacc
import concourse.tile as tile
import concourse.mybir as mybir
from concourse.bass_utils import run_bass_kernel_spmd

B, C, HH, WW = 4, 256, 64, 64
N = HH * WW          # 4096
CQK = 32
IH = N // 2          # 2048 query rows per core
NCORES = 8
NG = 16              # groups of 2 key 128-chunks per ib
NIB = 4              # query blocks of 512 per core-branch

F32 = mybir.dt.float32
F32R = mybir.dt.float32r
BF16 = mybir.dt.bfloat16
AF = mybir.ActivationFunctionType
ALU = mybir.AluOpType

# column offsets inside the packed f32 weight tile [128, 1536]
WQ_OFF, WK_OFF, WV_OFF, WCX_OFF = 0, 256, 512, 1024


def build_program(nc, tc):
    # ---- DRAM I/O ------------------------------------------------------
    dram = {}
    for name, shape, dt in [
        ("xf1", [2, 128, N], F32R), ("xf2", [2, 128, N], F32R),
        ("wpk", [128, 1536], F32R), ("wca", [128, 512], BF16),
        ("bq", [128, 1], F32), ("bce", [128, 2], F32),
    ]:
        dram[name] = nc.dram_tensor(name, shape, dt, kind="ExternalInput").ap()
    out_d = nc.dram_tensor("out", [2, IH], F32, kind="ExternalOutput").ap()

    import contextlib
    with contextlib.ExitStack() as ctx:
        persist = ctx.enter_context(tc.tile_pool(name="persist", bufs=1))

        wpk_sb = persist.tile([128, 1536], F32R, tag="wpk")
        wca_sb = persist.tile([128, 512], BF16, tag="wca")
        bq_sb = persist.tile([128, 1], F32, tag="bq")
        bce_sb = persist.tile([128, 2], F32, tag="bce")
        ones_bf = persist.tile([128, 1], BF16, tag="onesb")
        ones_row = persist.tile([1, 512], BF16, tag="onesr")

        # weights on the gpsimd DMA queue, xf kc0 chunks on sync queue
        nc.gpsimd.dma_start(out=wpk_sb, in_=dram["wpk"])
        nc.gpsimd.dma_start(out=wca_sb, in_=dram["wca"])
        nc.gpsimd.dma_start(out=bq_sb, in_=dram["bq"])
        nc.gpsimd.dma_start(out=bce_sb, in_=dram["bce"])
        nc.vector.memset(ones_bf, 1.0)
        nc.vector.memset(ones_row, 1.0)

        x1i_sb = [persist.tile([128, IH], F32R, tag=f"x1i{kc}",
                               name=f"x1i{kc}") for kc in range(2)]
        q4_sb = [persist.tile([128, IH], F32R, tag=f"q{i}", name=f"q{i}")
                 for i in range(2)]
        k4_sb = [[persist.tile([128, IH], F32R, tag=f"k{i}{h}",
                               name=f"k{i}{h}") for h in range(2)]
                 for i in range(2)]
        vT_sb = [[persist.tile([128, 16 * C], BF16, tag=f"vt{i}{h}",
                               name=f"vt{i}{h}") for h in range(2)]
                 for i in range(2)]

        # ---- phase 1: projections (DMA/PE/ACT/DVE pipelined) ----------
        with tc.tile_pool(name="proj_sb", bufs=1) as proj_sb, \
             tc.tile_pool(name="ps_proj", bufs=1, space="PSUM") as ps_proj:

            xfd = {0: "xf1", 1: "xf2"}
            for xi, jh in [(0, 0), (1, 0), (0, 1), (1, 1)]:
                xf_t = proj_sb.tile([128, 2, IH], F32R, tag="xf", bufs=2,
                                    name=f"xf{xi}{jh}")
                half = IH // 2
                dq = (nc.sync, nc.gpsimd, nc.scalar)
                ti = 2 * xi + jh
                for ci, (kc, hf) in enumerate(
                        ((0, 0), (0, 1), (1, 0), (1, 1))):
                    dq[(ci + ti) % 3].dma_start(
                        out=xf_t[:, kc, bass.ds(hf * half, half)],
                        in_=dram[xfd[xi]][kc][:, bass.ds(jh * IH + hf * half,
                                                         half)])
                if jh == 0:
                    # queries live in the local first half
                    for i4 in range(4):
                        sl = bass.ts(i4, 512)
                        qp = ps_proj.tile([128, 512], F32, tag="kq", bufs=3,
                                          name="qp")
                        for kc in range(2):
                            nc.tensor.matmul(
                                qp, wpk_sb[:, bass.ds(WQ_OFF + kc * 128, 128)],
                                xf_t[:, kc, sl],
                                start=(kc == 0), stop=(kc == 1))
                        nc.scalar.activation(q4_sb[xi][:, sl], qp, AF.Identity,
                                             bias=bq_sb)
                    if xi == 0:
                        for kc in range(2):
                            nc.scalar.activation(x1i_sb[kc], xf_t[:, kc, :],
                                                 AF.Copy)
                for jb in range(4):
                    sl = bass.ts(jb, 512)
                    kp = ps_proj.tile([128, 512], F32, tag="kq", bufs=3,
                                      name="kp")
                    for kc in range(2):
                        nc.tensor.matmul(
                            kp, wpk_sb[:, bass.ds(WK_OFF + kc * 128, 128)],
                            xf_t[:, kc, sl],
                            start=(kc == 0), stop=(kc == 1))
                    nc.scalar.activation(k4_sb[xi][jh][:, sl], kp, AF.Copy)
                for s2 in range(8):
                    vtp = ps_proj.tile([128, 512], F32, tag="vt", bufs=3,
                                       name="vtp")
                    for s in range(2):
                        jcl = 2 * s2 + s
                        for kc in range(2):
                            nc.tensor.matmul(
                                vtp[:, bass.ds(s * 256, 256)],
                                xf_t[:, kc, bass.ds(jcl * 128, 128)],
                                wpk_sb[:, bass.ds(WV_OFF + kc * 256, 256)],
                                start=(kc == 0), stop=(kc == 1))
                    nc.vector.tensor_copy(
                        vT_sb[xi][jh][:, bass.ds(s2 * 512, 512)], vtp)

        # ---- phase 2: attention + fused combine -----------------------
        with tc.tile_pool(name="attn_sb", bufs=1) as asb, \
             tc.tile_pool(name="ps_attn", bufs=1, space="PSUM") as psp:

            seq = [(br, ib) for br in range(2) for ib in range(4)]
            st = {k: {"stp": {}, "est": {}} for k in seq}

            def emit_score(key, g):
                br, ib = key
                isl = bass.ts(ib, 512)
                jh, jl = divmod(g, 8)
                stp = psp.tile([128, 2, 512], F32, tag="stp", bufs=2,
                               name=f"stp{br}{ib}g{g}")
                for t in range(2):
                    jcl = 2 * jl + t
                    nc.tensor.matmul(
                        stp[:, t, :],
                        k4_sb[br][jh][32 * t:32 * (t + 1),
                                      bass.ds(jcl * 128, 128)],
                        q4_sb[br][32 * t:32 * (t + 1), isl],
                        start=True, stop=True, tile_position=(32 * t, 0))
                st[key]["stp"][g] = stp

            def emit_exp(key, g):
                stp = st[key]["stp"].pop(g)
                est = asb.tile([128, 2, 512], BF16, tag="est", bufs=4,
                               name=f"est{key[0]}{key[1]}g{g}")
                nc.scalar.activation(est.rearrange("p a n -> p (a n)"),
                                     stp.rearrange("p a n -> p (a n)"), AF.Exp)
                st[key]["est"][g] = est

            deferred = None  # (outw, absb0, absb1, br, isl)

            def emit_deferred():
                nonlocal deferred
                outw, ab0, ab1, obr, oisl = deferred
                for c2, ab in ((0, ab0), (1, ab1)):
                    nc.tensor.matmul(outw[0:1, :], ones_bf, ab,
                                     start=(c2 == 0), stop=(c2 == 1))
                osb = asb.tile([1, 512], F32, tag="osb", bufs=2, name="osb")
                nc.vector.tensor_copy(osb, outw[0:1, :])
                nc.sync.dma_start(out=out_d[obr:obr + 1, oisl], in_=osb)
                deferred = None

            for idx, key in enumerate(seq):
                br, ib = key
                isl = bass.ts(ib, 512)
                vT = vT_sb[1 - br]
                attp = [psp.tile([128, 512], F32, tag="attp", bufs=2,
                                 name=f"attp{br}{ib}c{c2}")
                        for c2 in range(2)]
                rp = psp.tile([128, 512], F32, tag="rp", bufs=2,
                              name=f"rp{br}{ib}")
                for g in (0, 1):
                    if g not in st[key]["stp"] and g not in st[key]["est"]:
                        emit_score(key, g)

                for g in range(NG):
                    if g == 2 and deferred is not None:
                        emit_deferred()
                    if g not in st[key]["est"]:
                        emit_exp(key, g)
                    if g < NG - 2 and (g + 2) not in st[key]["stp"]:
                        emit_score(key, g + 2)
                    est = st[key]["est"][g]
                    jh, jl = divmod(g, 8)
                    for t in range(2):
                        jcl = 2 * jl + t
                        for c2 in range(2):
                            nc.tensor.matmul(
                                attp[c2],
                                vT[jh][:, bass.ds(jcl * 256 + c2 * 128, 128)],
                                est[:, t, :],
                                start=(g == 0 and t == 0),
                                stop=(g == NG - 1 and t == 1))
                    for t in range(2):
                        nc.tensor.matmul(
                            rp[32 * t:32 * t + 1, :], ones_bf, est[:, t, :],
                            start=(g == 0), stop=(g == NG - 1),
                            tile_position=(0, 32 * t))
                    del st[key]["est"][g]

                # ---- tail ---------------------------------------------
                nxt = seq[idx + 1] if idx + 1 < len(seq) else None
                rt = asb.tile([1, 512], F32, tag="rt", bufs=2, name="rt")
                nc.vector.tensor_copy(rt, rp[0:1, :])
                rsum = asb.tile([1, 512], F32, tag="rsum", bufs=2,
                                name="rsum")
                nc.vector.tensor_add(rsum, rt, rp[32:33, :])

                if nxt is not None:
                    emit_score(nxt, 0)
                    emit_score(nxt, 1)
                    emit_exp(nxt, 0)

                # c2=0 combine accumulator from the rp ring: free now
                cp0 = psp.tile([128, 512], F32, tag="rp", bufs=2,
                               name=f"cp0_{br}{ib}")
                for kc in range(2):
                    nc.tensor.matmul(
                        cp0, wpk_sb[:, bass.ds(WCX_OFF + kc * 256, 128)],
                        x1i_sb[kc][:, isl], start=(kc == 0), stop=False)

                # c2=1 combine accumulator borrows half an stp-ring tile
                cp1_t = psp.tile([128, 2, 512], F32, tag="stp", bufs=2,
                                 name=f"cp1_{br}{ib}")
                cp1 = cp1_t[:, 0, :]
                for kc in range(2):
                    nc.tensor.matmul(
                        cp1, wpk_sb[:, bass.ds(WCX_OFF + kc * 256 + 128, 128)],
                        x1i_sb[kc][:, isl], start=(kc == 0), stop=False)

                rr = asb.tile([1, 512], F32, tag="rr", bufs=2, name="rr")
                nc.vector.reciprocal_approx_fast(rr, rsum)
                rrb = asb.tile([1, 512], BF16, tag="rrb", bufs=2, name="rrb")
                nc.vector.tensor_copy(rrb, rr)
                rbc = psp.tile([128, 512], F32, tag="rp", bufs=2,
                               name=f"rbc{br}{ib}")
                nc.tensor.matmul(rbc, ones_row[:, 0:128], rrb,
                                 start=True, stop=True)

                if nxt is not None:
                    emit_exp(nxt, 1)

                rbcs = asb.tile([128, 512], F32, tag="rbcs", bufs=2,
                                name="rbcs")
                nc.vector.tensor_copy(rbcs, rbc)
                attst = []
                for c2 in range(2):
                    a = asb.tile([128, 512], BF16, tag="attst", bufs=4,
                                 name=f"attst{c2}")
                    nc.vector.tensor_mul(a, attp[c2], rbcs)
                    attst.append(a)
                for c2, cp in ((0, cp0), (1, cp1)):
                    for kcA in range(2):
                        nc.tensor.matmul(
                            cp, wca_sb[:, bass.ds(kcA * 256 + c2 * 128, 128)],
                            attst[kcA], start=False, stop=(kcA == 1))
                absb = []
                for c2, cp in ((0, cp0), (1, cp1)):
                    ab = asb.tile([128, 512], BF16, tag="absb", bufs=4,
                                  name=f"absb{c2}")
                    nc.scalar.activation(ab, cp, AF.Abs,
                                         bias=bce_sb[:, c2:c2 + 1])
                    absb.append(ab)
                outw = psp.tile([128, 512], F32, tag="rp", bufs=2,
                                name=f"outw{br}{ib}")
                deferred = (outw, absb[0], absb[1], br, isl)
                if nxt is None:
                    emit_deferred()


_NC_CACHE = {}


def _get_nc():
    if "nc" not in _NC_CACHE:
        nc = bacc.Bacc("TRN2", debug=False, enable_asserts=False,
                       target_bir_lowering=False, enable_partition_id=False)
        with tile.TileContext(nc) as tc:
            build_program(nc, tc)
        nc.compile()
        _NC_CACHE["nc"] = nc
    return _NC_CACHE["nc"]


def host_inputs(x1, x2, Wq, bq, Wk, bk, Wv, bv, Wc, bc):
    """Build the 8 per-core input maps (host-side sharding/layout only)."""
    f = np.float32
    x1 = np.asarray(x1, f); x2 = np.asarray(x2, f)
    Wq = np.asarray(Wq, f); bq = np.asarray(bq, f)
    Wk = np.asarray(Wk, f)
    Wv = np.asarray(Wv, f); bv = np.asarray(bv, f)
    Wc = np.asarray(Wc, f); bc = np.asarray(bc, f)

    wqt = np.tile(Wq, (4, 1)).T.reshape(128, 2, 128)   # [cin, kc, m]
    wkt = np.tile(Wk, (4, 1)).T.reshape(128, 2, 128)
    wvt = Wv.T.reshape(128, 2, 256)
    WcT = np.ascontiguousarray(Wc.T)                    # [512, 256]
    wcxt = WcT[:C].reshape(128, 2, 256)
    wpk = np.empty((128, 1536), f)
    # packed layouts are [2, 128, cols] kc-major along columns
    wpk[:, WQ_OFF:WQ_OFF + 256] = np.concatenate(
        [np.tile(Wq, (4, 1)).T.reshape(2, 128, 128)[kc] for kc in range(2)],
        axis=1)
    wpk[:, WK_OFF:WK_OFF + 256] = np.concatenate(
        [np.tile(Wk, (4, 1)).T.reshape(2, 128, 128)[kc] for kc in range(2)],
        axis=1)
    wpk[:, WV_OFF:WV_OFF + 512] = np.concatenate(
        [Wv.T.reshape(2, 128, 256)[kc] for kc in range(2)], axis=1)
    wpk[:, WCX_OFF:WCX_OFF + 512] = np.concatenate(
        [WcT[:C].reshape(2, 128, 256)[kc] for kc in range(2)], axis=1)

    WcaT = WcT[C:]                                      # [256, 256]
    wca = np.concatenate([WcaT[:128], WcaT[128:]],
                         axis=1).astype(ml_dtypes.bfloat16)  # [128, 512]
    bq4 = np.tile(bq, 4).reshape(128, 1).copy()
    bce = (bc + Wc[:, C:] @ bv).astype(f)               # [256]
    bcet = bce.reshape(2, 128).T.copy()

    in_maps = []
    for core in range(NCORES):
        b, h = divmod(core, 2)
        xf1 = x1[b].reshape(C, N)
        xf2 = x2[b].reshape(C, N)
        if h == 1:  # local-half permute: own query half first
            xf1 = np.concatenate([xf1[:, IH:], xf1[:, :IH]], axis=1)
            xf2 = np.concatenate([xf2[:, IH:], xf2[:, :IH]], axis=1)
        in_maps.append({
            "xf1": np.ascontiguousarray(xf1.reshape(2, 128, N)),
            "xf2": np.ascontiguousarray(xf2.reshape(2, 128, N)),
            "wpk": wpk, "wca": wca, "bq": bq4, "bce": bcet,
        })
    return in_maps


def assemble(results):
    """results: list of 8 dicts with 'out' [2, IH] -> (out1, out2) full."""
    outs = []
    for row in range(2):
        full = np.empty((B, 1, HH, WW), np.float32)
        for b in range(B):
            half0 = results[2 * b]["out"][row]
            half1 = results[2 * b + 1]["out"][row]
            full[b, 0] = np.concatenate([half0, half1]).reshape(HH, WW)
        outs.append(full)
    return outs[0], outs[1]


def kernel(x1, x2, Wq, bq, Wk, bk, Wv, bv, Wc, bc):
    in_maps = host_inputs(x1, x2, Wq, bq, Wk, bk, Wv, bv, Wc, bc)
    nc = _get_nc()
    res = run_bass_kernel_spmd(nc, in_maps, core_ids=list(range(NCORES)))
    return assemble(res.results)
